# revision 1
# baseline (speedup 1.0000x reference)
"""GCNN-alpha (gnn_message_passing) Trainium2 kernel.

Full-input contract: kernel(**inputs) takes the unsharded numpy inputs and
returns the [B, F_OUT] output. Internally:
  - Shards the graph-batch axis B=64 across 8 NeuronCores (8 graphs/core).
  - Host does integer-only edge preprocessing: per destination-row source
    lists with multiplicities (rows of S^T), packed/padded per partition.
  - On device, gpsimd.local_scatter builds S^T = (alpha*A + (1-alpha)*I)^T
    tiles directly in SBUF.
  - The two conv layers run as dense fp16 matmuls with fp32 PSUM
    accumulation, Horner's scheme out = P0 + S@(P1 + S@P2), P_k = Y@W_k,
    with a transpose-free structure that alternates matmul orientation:
      layer input Yt   [f=128 part, n=512 free]   (transposed)
      P0t  = lhsT W0, rhs Yt                      (1 mm, N=512, PSUM)
      P1u  = lhsT Yt-blk, rhs W1      per m-blk   (4 mm, N=128, PSUM)
      P2u  = lhsT Yt-blk, rhs W2      per m-blk   (4 mm, N=128, PSUM)
      u2   = copy P2u -> SBUF
      P1u += S@u2 via lhsT=S^T-blk, rhs=u2-blk    (16 mm, N=128)
      u1   = copy P1u -> SBUF
      P0t += (S@u1)^T via lhsT=u1-blk, rhs=S^T    (4 mm, N=512)
      Zt   = ACT ReLU(P0t + b)  (bias per-partition, fp16 out)
  - Max-pool is a free-axis reduce; readout is one matmul + K=1 bias mm.
"""

import os

import numpy as np
import ml_dtypes

import bass_rust
import concourse.bacc as bacc
import concourse.bass as bass
import concourse.tile as tile
from concourse import library_config, mybir
from concourse.bass_utils import run_bass_kernel_spmd

_add_dep = bass_rust.add_dep_helper

# Problem dims (hardcoded per spec).
B, n = 64, 512
F_IN, F_HID, F_OUT = 128, 128, 64
M = 8            # NeuronCores
GPC = B // M     # graphs per core
PB = n // 128    # 128-row partition blocks per graph
SPG = 2          # local_scatter calls per graph (num_elems=1024 each)
BPS = PB // SPG  # partition blocks covered per scatter call

_DT_NAME = os.environ.get("GNN_DT", "float16")
_REPEAT = int(os.environ.get("GNN_REPEAT", "1"))

_NP_DT = {
    "float16": np.float16,
    "bfloat16": ml_dtypes.bfloat16,
}[_DT_NAME]
_MY_DT = {
    "float16": mybir.dt.float16,
    "bfloat16": mybir.dt.bfloat16,
}[_DT_NAME]

_BUILD_CACHE = {}


def _build(k2: int):
    """Emit the per-core Bass program (identical on all 8 cores).
    k2 = num_idxs per local_scatter call (covers BPS row-blocks)."""
    DT = _MY_DT
    f32 = mybir.dt.float32
    nc = bacc.Bacc("TRN2", target_bir_lowering=False, debug=False,
                   num_devices=M)

    xt_d = nc.dram_tensor("xt", [128, GPC * n], DT, kind="ExternalInput")
    sidx_d = nc.dram_tensor("sidx", [128, GPC * SPG * k2], mybir.dt.int16,
                            kind="ExternalInput")
    sval_d = nc.dram_tensor("sval", [128, GPC * SPG * k2], DT,
                            kind="ExternalInput")
    w1_d = nc.dram_tensor("w1", [128, 3 * F_HID], DT, kind="ExternalInput")
    w2_d = nc.dram_tensor("w2", [128, 3 * F_HID], DT, kind="ExternalInput")
    wout_d = nc.dram_tensor("wout", [128, F_OUT], DT, kind="ExternalInput")
    b1_d = nc.dram_tensor("b1", [128, 1], f32, kind="ExternalInput")
    b2_d = nc.dram_tensor("b2", [128, 1], f32, kind="ExternalInput")
    bout_d = nc.dram_tensor("bout", [1, F_OUT], DT, kind="ExternalInput")
    out_d = nc.dram_tensor("out", [GPC, F_OUT], f32, kind="ExternalOutput")

    with tile.TileContext(nc) as tc:
        with (
            tc.tile_pool(name="consts", bufs=1) as consts,
            tc.tile_pool(name="st", bufs=4) as st_pool,
            tc.tile_pool(name="act", bufs=4) as act_pool,
            tc.tile_pool(name="u", bufs=8) as u_pool,
            tc.tile_pool(name="pp", bufs=2, space="PSUM") as pp,
        ):
            w1_t = consts.tile([128, 3 * F_HID], DT)
            nc.sync.dma_start(w1_t[:], w1_d.ap())
            w2_t = consts.tile([128, 3 * F_HID], DT)
            nc.sync.dma_start(w2_t[:], w2_d.ap())
            wout_t = consts.tile([128, F_OUT], DT)
            nc.sync.dma_start(wout_t[:], wout_d.ap())
            b1_t = consts.tile([128, 1], f32)
            nc.sync.dma_start(b1_t[:], b1_d.ap())
            b2_t = consts.tile([128, 1], f32)
            nc.sync.dma_start(b2_t[:], b2_d.ap())
            bout_t = consts.tile([1, F_OUT], DT)
            nc.sync.dma_start(bout_t[:], bout_d.ap())
            ones_t = consts.tile([1, GPC], DT)
            nc.vector.memset(ones_t[:], 1.0)

            # Per-graph input DMAs: graph 0's scatter/matmuls start as
            # soon as its own slices land (one-shot pipeline fill).
            sidx_t = consts.tile([128, GPC * SPG * k2], mybir.dt.int16)
            sval_t = consts.tile([128, GPC * SPG * k2], DT)
            xt_t = consts.tile([128, GPC * n], DT)
            for g in range(GPC):
                sl = slice(g * SPG * k2, (g + 1) * SPG * k2)
                nc.sync.dma_start(sidx_t[:, sl], sidx_d.ap()[:, sl])
                nc.sync.dma_start(sval_t[:, sl], sval_d.ap()[:, sl])
                xsl = slice(g * n, (g + 1) * n)
                nc.sync.dma_start(xt_t[:, xsl], xt_d.ap()[:, xsl])

            nc.gpsimd.load_library(library_config.local_scatter)

            for rep in range(_REPEAT):
                pooledT = consts.tile([128, GPC], DT, tag="pooled")
                for g in range(GPC):
                    # Build S^T for this graph: SPG scatters of [128, BPS*n].
                    st_g = st_pool.tile([128, PB * n], DT, tag="st")
                    for s in range(SPG):
                        off = (g * SPG + s) * k2
                        nc.gpsimd.local_scatter(
                            out_ap=st_g[:, s * BPS * n:(s + 1) * BPS * n],
                            data_ap=sval_t[:, off:off + k2],
                            idxs_ap=sidx_t[:, off:off + k2],
                            channels=128,
                            num_elems=BPS * n,
                            num_idxs=k2,
                        )

                    yt = xt_t[:, g * n:(g + 1) * n]
                    for layer in range(2):
                        w_t = w1_t if layer == 0 else w2_t
                        b_t = b1_t if layer == 0 else b2_t
                        p0 = pp.tile([128, n], f32, tag="P0")   # transposed
                        p1 = pp.tile([128, n], f32, tag="P1")   # untransposed
                        p2 = pp.tile([128, n], f32, tag="P2")   # untransposed
                        # P2u untransposed first (so the u2 copy can start
                        # as early as possible): per m-block, lhsT=Yt-blk.
                        # (start=True clears has_written for the WHOLE psum
                        # bank, but p2 is never accumulated into afterwards,
                        # so per-region start=True groups are fine here.)
                        for mb in range(PB):
                            ytb = yt[:, mb * 128:(mb + 1) * 128]
                            nc.tensor.matmul(
                                p2[:, mb * 128:(mb + 1) * 128], ytb,
                                w_t[:, 256:384],
                                start=True, stop=True)
                        # u2 copy on DVE (faster per-op; it's on hop1's
                        # critical path), u1 on ACT balances engine load
                        u2 = u_pool.tile([128, n], DT, tag="u")
                        nc.vector.tensor_copy(u2[:], p2[:])
                        # P0t = W0^T @ Yt (transposed out, one N=512 mm);
                        # PE does this while ACT copies u2.
                        nc.tensor.matmul(p0[:], w_t[:, 0:128], yt,
                                         start=True, stop=False)
                        # P1u + hop1 (P1u += S @ u2): one consecutive chain
                        # per m-block region, led by the W-product with
                        # start=True. A later chain's start=True clears the
                        # whole bank's has_written, so chains must not
                        # interleave -> explicit ordering edges.
                        prev_last = None
                        for mb in range(PB):
                            ytb = yt[:, mb * 128:(mb + 1) * 128]
                            mm = nc.tensor.matmul(
                                p1[:, mb * 128:(mb + 1) * 128], ytb,
                                w_t[:, 128:256],
                                start=True, stop=False,
                                skip_group_check=True)
                            if prev_last is not None:
                                _add_dep(mm.ins, prev_last.ins, sync=False,
                                         reason="psum bank chain order")
                            for kb in range(PB):
                                mm = nc.tensor.matmul(
                                    p1[:, mb * 128:(mb + 1) * 128],
                                    st_g[:, kb * n + mb * 128:
                                         kb * n + (mb + 1) * 128],
                                    u2[:, kb * 128:(kb + 1) * 128],
                                    start=False,
                                    stop=(kb == PB - 1),
                                    skip_group_check=True)
                            prev_last = mm
                        # u1 copy also on ACT: DVE was the constraint
                        # (measured 29.3us vs 40.9us with u1 on DVE).
                        u1 = u_pool.tile([128, n], DT, tag="u")
                        nc.scalar.copy(u1[:], p1[:])
                        # hop2: P0t += (S @ u1)^T (transposed out, N=512)
                        for kb in range(PB):
                            nc.tensor.matmul(
                                p0[:], u1[:, kb * 128:(kb + 1) * 128],
                                st_g[:, kb * n:(kb + 1) * n],
                                start=False, stop=(kb == PB - 1),
                                skip_group_check=True)
                        # ReLU(out + b) straight off PSUM, cast to DT
                        zt = act_pool.tile([128, n], DT, tag="z")
                        nc.scalar.activation(
                            zt[:], p0[:], mybir.ActivationFunctionType.Relu,
                            bias=b_t[:])
                        yt = zt[:]

                    nc.vector.tensor_reduce(
                        pooledT[:, g:g + 1], yt, axis=mybir.AxisListType.X,
                        op=mybir.AluOpType.max)

                # Readout: out = pooled @ Wout + bout
                po = pp.tile([GPC, F_OUT], f32, tag="P0")
                nc.tensor.matmul(po[:], pooledT[:], wout_t[:],
                                 start=True, stop=False)
                nc.tensor.matmul(po[:], ones_t[:], bout_t[:],
                                 start=False, stop=True,
                                 skip_group_check=True)
                out_sb = consts.tile([GPC, F_OUT], f32, tag="osb")
                nc.vector.tensor_copy(out_sb[:], po[:])
                nc.sync.dma_start(out_d.ap(), out_sb[:])

    nc.compile()
    return nc


def _prep_scatter(edge_index: np.ndarray, alpha: float):
    """Integer-only edge prep. Returns per-scatter-call packed index/value
    arrays [B*n partition-rows grouped, SPG, k2] with trailing -1 padding.

    Row r of S^T (= dst node) has entries {src: alpha*mult} plus
    (1-alpha) added at column r. Scatter call s for graph g covers row
    blocks [s*BPS, (s+1)*BPS); partition p handles rows
    {g*n + s*BPS*128 + j*128 + p for j in range(BPS)} with column offset
    j*n inside the [128, BPS*n] output tile.
    """
    src = edge_index[0].astype(np.int64)
    dst = edge_index[1].astype(np.int64)
    g = src // n
    src_l = src - g * n
    dst_l = dst - g * n
    ekey = (g * n + dst_l) * n + src_l
    ukey, ucnt = np.unique(ekey, return_counts=True)
    uval = alpha * ucnt.astype(np.float64)
    rows_all = np.arange(B * n, dtype=np.int64)
    dkey = rows_all * n + (rows_all % n)
    pos = np.searchsorted(ukey, dkey)
    pos_c = np.minimum(pos, len(ukey) - 1)
    hit = ukey[pos_c] == dkey
    uval[pos_c[hit]] += 1.0 - alpha
    allk = np.concatenate([ukey, dkey[~hit]])
    allv = np.concatenate([uval, np.full((~hit).sum(), 1.0 - alpha)])
    order = np.argsort(allk, kind="stable")
    allk = allk[order]
    allv = allv[order]

    row = allk // n          # global row id in [0, B*n)
    col = allk % n           # column within the row's graph
    # Map each entry to (graph, scatter-call s, partition p, block j).
    r_l = row % n
    gg = row // n
    s = r_l // (BPS * 128)
    j = (r_l // 128) % BPS
    p = r_l % 128
    # partition-slot key: for (gg, s, p): entries packed ordered by (j, col)
    slot = (gg * SPG + s) * 128 + p
    order2 = np.lexsort((col, j, slot))
    slot = slot[order2]
    scat_col = (j * n + col)[order2].astype(np.int16)
    vals = allv[order2]

    nslots = B * SPG * 128
    slot_start = np.searchsorted(slot, np.arange(nslots))
    rank = np.arange(len(slot)) - slot_start[slot]
    k_need = int(rank.max()) + 1
    k2 = max(64, (k_need + 15) // 16 * 16)

    idx_arr = np.full((nslots, k2), -1, np.int16)
    val_arr = np.zeros((nslots, k2), np.float64)
    idx_arr[slot, rank] = scat_col
    val_arr[slot, rank] = vals
    # [B*SPG, 128, k2] -> group: graph-major per core handled in _make_in_maps
    return idx_arr.reshape(B, SPG, 128, k2), val_arr.reshape(B, SPG, 128, k2), k2


def _make_in_maps(inputs, idx_arr, val_arr):
    # idx/val: [B, SPG, 128, k2] -> per-core partition-major
    # [128, GPC*SPG*k2] with free order (g, s, k).
    def core_scatter(arr, np_dt):
        a = np.ascontiguousarray(arr).reshape(M, GPC, SPG, 128, -1)
        return [np.ascontiguousarray(
            a[c].transpose(2, 0, 1, 3).reshape(128, -1)).astype(np_dt)
            for c in range(M)]

    sidx = core_scatter(idx_arr, np.int16)
    sval = core_scatter(val_arr, _NP_DT)

    X = np.asarray(inputs["X"], np.float32)
    w1_h = np.ascontiguousarray(
        np.asarray(inputs["W1"], np.float32).transpose(1, 0, 2)
        .reshape(128, 3 * F_HID)).astype(_NP_DT)
    w2_h = np.ascontiguousarray(
        np.asarray(inputs["W2"], np.float32).transpose(1, 0, 2)
        .reshape(128, 3 * F_HID)).astype(_NP_DT)
    wout_h = np.asarray(inputs["Wout"], np.float32).astype(_NP_DT)
    b1_h = np.asarray(inputs["b1"], np.float32).reshape(128, 1)
    b2_h = np.asarray(inputs["b2"], np.float32).reshape(128, 1)
    bout_h = np.asarray(inputs["bout"], np.float32).reshape(1, F_OUT).astype(_NP_DT)

    in_maps = []
    for c in range(M):
        xc = X[c * GPC * n:(c + 1) * GPC * n].reshape(GPC, n, F_IN)
        xt = np.ascontiguousarray(
            xc.transpose(2, 0, 1).reshape(128, GPC * n)).astype(_NP_DT)
        in_maps.append({
            "xt": xt,
            "sidx": sidx[c],
            "sval": sval[c],
            "w1": w1_h,
            "w2": w2_h,
            "wout": wout_h,
            "b1": b1_h,
            "b2": b2_h,
            "bout": bout_h,
        })
    return in_maps


def kernel(X, batch, edge_index, W1, b1, W2, b2, Wout, bout, alpha):
    alpha_f = float(np.asarray(alpha))
    idx_arr, val_arr, k2 = _prep_scatter(np.asarray(edge_index), alpha_f)
    in_maps = _make_in_maps(
        {"X": X, "W1": W1, "b1": b1, "W2": W2, "b2": b2,
         "Wout": Wout, "bout": bout}, idx_arr, val_arr)

    key = (k2, _DT_NAME, _REPEAT)
    if key not in _BUILD_CACHE:
        _BUILD_CACHE[key] = _build(k2)
    nc = _BUILD_CACHE[key]

    res = run_bass_kernel_spmd(nc, in_maps, core_ids=list(range(M)))
    return np.concatenate([res.results[c]["out"] for c in range(M)], axis=0)



# revision 23
# speedup vs baseline: 1.0619x; 1.0619x over previous
"""GCNN-alpha (gnn_message_passing) Trainium2 kernel, v2.

Full-input contract: kernel(**inputs) takes the unsharded numpy inputs and
returns the [B, F_OUT] float32 output. B=64 graphs are sharded 8-per-core
across 8 NeuronCores; each core runs an identical program.

Math: per graph, out = max_pool(ReLU-conv x2) @ Wout + bout with
conv(X) = ReLU(sum_k S^k X W_k + b), S = alpha*A + (1-alpha)*I.
Host folds alpha and the diagonal into the weights:
    A' hop form:  out = X@Wh0 + A@(X@Wh1 + A@(X@Wh2))   (per layer)
    Wh2 = a^2 W2,  Wh1 = a W1 + 2c a^2 W2,  Wh0 = W0 + c a W1 + c^2 a^2 W2
    with c=(1-a)/a  -- so the device-side sparse operand is the RAW integer
    adjacency A (multiplicities), exactly representable in fp8e4 (<=240).

Device design (per core, per layer; graphs pipelined with stage skew so the
PE never waits on PSUM->SBUF copies):
    A(g): P2u = Y@Wh2 per m-block             4 fp16 mm, FD=128   (PSUM p2)
    B(g): u2  = copy(P2u)   fp16              DVE
    C(g): per m-block: P1u(mb)=Y@Wh1 (start) then hop1 accumulate
          P1u(mb) += A-block^T.T @ u2-block   4x(1+4) mm, fp8xfp16 mixed
    D(g): u1  = copy(P1u)   L1: fp8 (ACT), L2: fp16 (ACT)
    E(g): P0t = Wh0^T @ Y (FD=512, start) then hop2 accumulate (transposed)
          L1: 2 fp8 DoubleRow mm (lhsT=u1-pairs, rhs=A^T-pairs, FD=512)
          L2: 4 fp16xfp8 mixed mm (lhsT=u1-block, rhs=A^T-rows, FD=512)
    F(g): L1: zt = ACT ReLU(P0t + b1) -> fp16 (next layer's Y)
          L2: DVE reduce-max over free axis straight off PSUM -> pooledT
              (max then bias+ReLU commute: b per-partition, monotone)
Emission order per layer: A0 A1 C0 A2 C1 E0 A3 C2 E1 ... C7 E6 E7 -- the
stage skew keeps PE busy while DVE/ACT copies drain.
Readout: pooled2 = ReLU(pooledT + b2); out = pooled2 @ Wout + bout.
A^T tiles (dense, int-valued fp8) are host-built and DMA-streamed; no gpsimd.
"""

import os

import numpy as np

import bass_rust
import concourse.bacc as bacc
import concourse.tile as tile
from concourse import mybir
from concourse.bass_utils import run_bass_kernel_spmd

_add_dep = bass_rust.add_dep_helper

# Problem dims (hardcoded per spec).
B, n = 64, 512
F_IN, F_HID, F_OUT = 128, 128, 64
M = 8            # NeuronCores
GPC = B // M     # graphs per core
PB = n // 128    # 128-row partition blocks per graph

_REPEAT = int(os.environ.get("GNN_REPEAT", "1"))

F16 = mybir.dt.float16
F8 = mybir.dt.float8e4
F32 = mybir.dt.float32
NP_F8 = mybir.dt.np(F8)   # ml_dtypes.float8_e4m3 (TRN e4m3, max 240)

_BUILD_CACHE = {}


def _build(repeat: int):
    """Emit the per-core Bass program (identical on all 8 cores)."""
    nc = bacc.Bacc("TRN2", target_bir_lowering=False, debug=False,
                   num_devices=M)

    xt_d = nc.dram_tensor("xt", [128, GPC * n], F16, kind="ExternalInput")
    at8_d = nc.dram_tensor("at8", [128, GPC * PB * n], F8,
                           kind="ExternalInput")
    # wpk packs [w1 | wout | w2] = [128, 384+64+384] fp16 (w2 last: needed
    # only at layer 2, but one DMA is cheaper than two on the const queue).
    wpk_d = nc.dram_tensor("wpk", [128, 6 * F_HID + F_OUT], F16,
                           kind="ExternalInput")
    b12_d = nc.dram_tensor("b12", [128, 2], F32, kind="ExternalInput")
    bout_d = nc.dram_tensor("bout", [1, F_OUT], F16, kind="ExternalInput")
    out_d = nc.dram_tensor("out", [GPC, F_OUT], F32, kind="ExternalOutput")

    with tile.TileContext(nc) as tc:
        with (
            tc.tile_pool(name="consts", bufs=1) as consts,
            tc.tile_pool(name="z", bufs=1) as z_pool,
            tc.tile_pool(name="u2p", bufs=3) as u2_pool,
            tc.tile_pool(name="u1p", bufs=3) as u1_pool,
            tc.tile_pool(name="pp", bufs=2, space="PSUM") as pp,
            tc.tile_pool(name="ppo", bufs=1, space="PSUM") as ppo,
        ):
            # Three parallel DMA issue queues: xt slices on SP, A^T slices on
            # the Activation hwdge, consts on the (otherwise idle) gpsimd
            # SWDGE — so graph 0's inputs and the weights all land ~2us in.
            xt_t = consts.tile([128, GPC * n], F16)
            at8_t = consts.tile([128, GPC * PB * n], F8)
            wpk_t = consts.tile([128, 6 * F_HID + F_OUT], F16)
            b12_t = consts.tile([128, 2], F32)
            bout_t = consts.tile([1, F_OUT], F16)
            # Input DMAs ride the SP hwdge and the gpsimd SWDGE only — the
            # Activation queue must stay free of DMA issues (its sequencer is
            # strict FIFO, so a DMA issue there would block the u1 copies
            # behind it). Interleave so each slice lands just before its
            # consumer stage.
            def xsl_(g):
                return slice(g * n, (g + 1) * n)

            def asl_(g):
                return slice(g * PB * n, (g + 1) * PB * n)

            def dma_x(eng, g):
                eng.dma_start(xt_t[:, xsl_(g)], xt_d.ap()[:, xsl_(g)])

            def dma_a(eng, g):
                eng.dma_start(at8_t[:, asl_(g)], at8_d.ap()[:, asl_(g)])

            # A(u0) needs wpk+xt0: land them in parallel on the two queues.
            nc.sync.dma_start(wpk_t[:], wpk_d.ap())
            dma_x(nc.gpsimd, 0)
            dma_a(nc.gpsimd, 0)
            dma_x(nc.sync, 1)
            dma_a(nc.sync, 1)
            nc.sync.dma_start(b12_t[:], b12_d.ap())
            dma_x(nc.gpsimd, 2)
            dma_a(nc.gpsimd, 2)
            for g in range(3, GPC):
                dma_a((nc.sync if g % 2 == 1 else nc.gpsimd), g)
                dma_x((nc.gpsimd if g % 2 == 1 else nc.sync), g)
            nc.sync.dma_start(bout_t[:], bout_d.ap())

            w1_t = wpk_t[:, 0:3 * F_HID]
            wout_t = wpk_t[:, 3 * F_HID:3 * F_HID + F_OUT]
            w2_t = wpk_t[:, 3 * F_HID + F_OUT:6 * F_HID + F_OUT]
            b1_t = b12_t[:, 0:1]
            b2_t = b12_t[:, 1:2]
            ones_t = consts.tile([1, GPC], F16)
            nc.vector.memset(ones_t[:], 1.0)

            # zt tiles persist across the layer boundary: one per graph.
            zt = [z_pool.tile([128, n], F16, tag=f"z{g}", name=f"zt{g}")
                  for g in range(GPC)]

            def at_blk(g, kb):
                """A^T rows block kb of graph g: [128, n] (srcs kb*128+p)."""
                off = g * PB * n + kb * n
                return at8_t[:, off:off + n]

            def at_pairs(g, kb2):
                """A^T row pair-block: [128, 2, n] for DoubleRow rhs."""
                off = g * PB * n + kb2 * 2 * n
                return at8_t[:, off:off + 2 * n].rearrange(
                    "p (two f) -> p two f", two=2)

            # One flat software pipeline over all (rep, layer, graph) units
            # with a 3-stage skew: A feeds C feeds E, so the PE never waits
            # on the DVE/ACT PSUM->SBUF copies and the pipeline fills/drains
            # exactly once for the whole program.
            pooledT = [consts.tile([128, GPC], F16, tag=f"pooled{i}",
                                   name=f"pooledT{i}") for i in range(2)]
            st = {}   # unit -> dict of live tiles

            def uw(layer):
                return w1_t if layer == 0 else w2_t

            def uyt(layer, g):
                return xt_t[:, g * n:(g + 1) * n] if layer == 0 else zt[g][:]

            def stage_A(u):
                rep, layer, g = u
                s = st[u] = {}
                # P2 lives A(u)..C(u) = 2 pipeline units -> 3 generations
                s["p2"] = pp.tile([128, n], F32, tag="P2", name="p2", bufs=3)
                yt = uyt(layer, g)
                for mb in range(PB):
                    nc.tensor.matmul(
                        s["p2"][:, mb * 128:(mb + 1) * 128],
                        yt[:, mb * 128:(mb + 1) * 128],
                        uw(layer)[:, 256:384], start=True, stop=True,
                        skip_group_check=True)
                s["u2"] = u2_pool.tile([128, n], F16, tag="u2", name="u2")
                nc.vector.tensor_copy(s["u2"][:], s["p2"][:])

            def stage_C(u):
                # Per m-block chain: W1-product (start) + 4 hop mms.
                # Chains must not interleave within the p1 bank (a later
                # start=True clears the bank's has_written), so add explicit
                # ordering edges between chains.
                rep, layer, g = u
                s = st[u]
                s["p1"] = pp.tile([128, n], F32, tag="P1", name="p1")
                yt = uyt(layer, g)
                prev_last = None
                for mb in range(PB):
                    mm = nc.tensor.matmul(
                        s["p1"][:, mb * 128:(mb + 1) * 128],
                        yt[:, mb * 128:(mb + 1) * 128],
                        uw(layer)[:, 128:256], start=True, stop=False,
                        skip_group_check=True)
                    if prev_last is not None:
                        _add_dep(mm.ins, prev_last.ins, sync=False,
                                 reason="psum chain order")
                    for kb in range(PB):
                        mm = nc.tensor.matmul(
                            s["p1"][:, mb * 128:(mb + 1) * 128],
                            at_blk(g, kb)[:, mb * 128:(mb + 1) * 128],
                            s["u2"][:, kb * 128:(kb + 1) * 128],
                            start=False, stop=(kb == PB - 1),
                            skip_group_check=True)
                    prev_last = mm
                s["u1"] = u1_pool.tile([128, n], F8 if layer == 0 else F16,
                                       tag=f"u1l{layer}", name="u1")
                nc.scalar.copy(s["u1"][:], s["p1"][:])

            def stage_E(u):
                rep, layer, g = u
                s = st[u]
                s["p0"] = pp.tile([128, n], F32, tag="P0", name="p0")
                nc.tensor.matmul(s["p0"][:], uw(layer)[:, 0:128],
                                 uyt(layer, g), start=True, stop=False)
                if layer == 0:
                    # hop2 via fp8 DoubleRow: 2 mm, contraction 256 each;
                    # lhsT = u1 pair-blocks, rhs = A^T pairs.
                    for kb2 in range(PB // 2):
                        nc.tensor.matmul(
                            s["p0"][:],
                            s["u1"][:, kb2 * 256:(kb2 + 1) * 256]
                            .rearrange("p (two f) -> p two f", two=2),
                            at_pairs(g, kb2),
                            start=False, stop=(kb2 == PB // 2 - 1),
                            perf_mode=mybir.MatmulPerfMode.DoubleRow,
                            skip_group_check=True)
                else:
                    for kb in range(PB):
                        nc.tensor.matmul(
                            s["p0"][:],
                            s["u1"][:, kb * 128:(kb + 1) * 128],
                            at_blk(g, kb),
                            start=False, stop=(kb == PB - 1),
                            skip_group_check=True)

            def stage_F(u):
                # ACT applies ReLU(p0 + b) -> fp16 into zt[g] (dead after
                # E(u) read it, safe to overwrite). Layer 2 then max-pools
                # the fp16 tile on DVE (2x mode) and, on the last graph,
                # triggers the rep's readout.
                rep, layer, g = u
                s = st.pop(u)
                nc.scalar.activation(
                    zt[g][:], s["p0"][:],
                    mybir.ActivationFunctionType.Relu,
                    bias=(b1_t if layer == 0 else b2_t)[:])
                if layer == 1:
                    nc.vector.tensor_reduce(
                        pooledT[rep % 2][:, g:g + 1], zt[g][:],
                        axis=mybir.AxisListType.X,
                        op=mybir.AluOpType.max)
                    if g == GPC - 1:
                        readout(rep)

            def readout(rep):
                # out = pooledT^T @ Wout + bout
                po = ppo.tile([GPC, F_OUT], F32, tag="PO", name="po")
                nc.tensor.matmul(po[:], pooledT[rep % 2][:], wout_t,
                                 start=True, stop=False)
                nc.tensor.matmul(po[:], ones_t[:], bout_t[:],
                                 start=False, stop=True,
                                 skip_group_check=True)
                out_sb = consts.tile([GPC, F_OUT], F32, tag="osb",
                                     name="out_sb")
                nc.vector.tensor_copy(out_sb[:], po[:])
                nc.sync.dma_start(out_d.ap(), out_sb[:])

            # Two-deep stage skew: A(i) | C(i-2) | E(i-4). Each PSUM->SBUF
            # copy gets ~2 units of PE work as slack, so neither the fill
            # nor the drain of the pipeline stalls the PE.
            units = [(rep, layer, g) for rep in range(_REPEAT)
                     for layer in range(2) for g in range(GPC)]
            NU = len(units)
            for i in range(NU + 4):
                if i < NU:
                    stage_A(units[i])
                if 2 <= i < NU + 2:
                    stage_C(units[i - 2])
                if i >= 4:
                    stage_E(units[i - 4])
                    stage_F(units[i - 4])

    nc.compile()
    return nc


def _prep(X, edge_index, W1, b1, W2, b2, Wout, bout, alpha):
    """Host-side input prep: dense per-graph A^T (fp8), folded weights."""
    a = float(np.asarray(alpha))

    src = np.asarray(edge_index[0], np.int64)
    dst = np.asarray(edge_index[1], np.int64)
    g = src // n
    # Contraction (partition) index = dst, output index = src: the reference
    # aggregates along OUT-edges (out[src] = sum_dst adj[src,dst] * x[dst]).
    key = (g * n + (dst - g * n)) * n + (src - g * n)
    counts = np.bincount(key, minlength=B * n * n).astype(np.float32)
    # [B, n, n] A^T (row=src, col=dst); SBUF layout [128, PB*n] per graph
    at = counts.reshape(B, PB, 128, n)

    def fold(W):
        # out = X@Wh0 + A@(X@Wh1 + A@(X@Wh2)) reproduces
        # sum_k (a*A + (1-a)*I)^k X W_k; polynomial in a (no division).
        W = np.asarray(W, np.float64)
        wh2 = a * a * W[2]
        wh1 = a * W[1] + 2 * a * (1 - a) * W[2]
        wh0 = W[0] + (1 - a) * W[1] + (1 - a) * (1 - a) * W[2]
        return np.concatenate([wh0, wh1, wh2], axis=1)  # [128, 384]

    wpk_h = np.concatenate(
        [fold(W1), np.asarray(Wout, np.float64), fold(W2)],
        axis=1).astype(np.float16)                     # [128, 384+64+384]
    b12_h = np.stack([np.asarray(b1, np.float64),
                      np.asarray(b2, np.float64)],
                     axis=1).astype(np.float32)        # [128, 2]
    bout_h = np.asarray(bout, np.float32).reshape(1, F_OUT).astype(np.float16)
    Xf = np.asarray(X, np.float32)

    in_maps = []
    for cix in range(M):
        xc = Xf[cix * GPC * n:(cix + 1) * GPC * n].reshape(GPC, n, F_IN)
        xt = np.ascontiguousarray(
            xc.transpose(2, 0, 1).reshape(128, GPC * n)).astype(np.float16)
        ac = at[cix * GPC:(cix + 1) * GPC]          # [GPC, PB, 128, n]
        at8 = np.ascontiguousarray(
            ac.transpose(2, 0, 1, 3).reshape(128, GPC * PB * n)).astype(NP_F8)
        in_maps.append({
            "xt": xt, "at8": at8,
            "wpk": wpk_h, "b12": b12_h, "bout": bout_h,
        })
    return in_maps


def kernel(X, batch, edge_index, W1, b1, W2, b2, Wout, bout, alpha):
    in_maps = _prep(X, edge_index, W1, b1, W2, b2, Wout, bout, alpha)
    if _REPEAT not in _BUILD_CACHE:
        _BUILD_CACHE[_REPEAT] = _build(_REPEAT)
    nc = _BUILD_CACHE[_REPEAT]
    res = run_bass_kernel_spmd(nc, in_maps, core_ids=list(range(M)))
    return np.concatenate([res.results[c]["out"] for c in range(M)], axis=0)
